# revision 64
# baseline (speedup 1.0000x reference)
import sys

for _p in ("/opt/trn_rl_repo",):
    if _p not in sys.path:
        sys.path.insert(0, _p)

import numpy as np

import concourse.bass as bass
import concourse.bacc as bacc
import concourse.mybir as mybir
from concourse.tile import TileContext
from concourse.bass_utils import run_bass_kernel_spmd

F32 = mybir.dt.float32
F16 = mybir.dt.float16
GE = mybir.AluOpType.is_ge
EQ = mybir.AluOpType.is_equal

B, N, C, H, W = 4, 4, 256, 100, 152
HH = 50                 # output rows per core (H split in halves)
WP = W + 2              # padded pitch
RB = 5                  # output rows per block
NBLK = HH // RB
SRC_LEN = (RB + 2) * WP          # source elements per block
REG = RB * WP                    # output-region elements per block (770)
BASE = WP + 1                    # offset of output (0,0) in the block source
XLEN = (HH + 2) * WP + 4         # 8012, incl. 4-elem tail slack
PBLK = SRC_LEN + 4               # block tile width (over-read slack)
YLEN = HH * WP                   # 7700
INV9C = 1.0 / (9.0 * C)
BIG = 9 * REG                    # merged 9-shift census width
TVL = REG + 2                    # vertical box-sum length (772)

SUBS = [(0, 512), (512, REG - 512)]
SUBS_TV = [(0, 512), (512, TVL - 512)]
SUBS_SRC = [(0, 512), (512, 512), (1024, SRC_LEN - 1024)]

_NC_CACHE = {}


def _win3(tile, base_off, dr):
    """3-level AP reading the 3 dc-shifted windows at row-shift dr:
    out[p, dc, j] = tile[p, base_off + (dr-1)*WP + (dc-1) + j]."""
    a = tile[:]
    return bass.AP(a.tensor, a.offset + base_off + (dr - 1) * WP - 1,
                   [[a.ap[0][0], 128], [1, 3], [1, REG]])


def _rep3(tile):
    """Broadcast a [128, REG] tile 3x along the free dim (step-0 AP)."""
    a = tile[:]
    return bass.AP(a.tensor, a.offset,
                   [[a.ap[0][0], 128], [0, 3], [1, REG]])


def build_nc():
    nc = bacc.Bacc(trn_type="TRN2")
    x1_h = nc.dram_tensor("x1", [C, XLEN], F16, kind="ExternalInput")
    x2_h = nc.dram_tensor("x2", [N, C, XLEN], F16, kind="ExternalInput")
    wft_h = nc.dram_tensor("wft", [C, C], F16, kind="ExternalInput")  # w_fuse.T [c,o]
    bf_h = nc.dram_tensor("bf", [C, 1], F32, kind="ExternalInput")
    y_h = nc.dram_tensor("y", [C, YLEN], F32, kind="ExternalOutput")

    with TileContext(nc) as tc:
        with (
            tc.tile_pool(name="const", bufs=1) as cpool,
            tc.tile_pool(name="pin1", bufs=3) as p1pool,
            tc.tile_pool(name="pin2", bufs=2) as p2pool,
            tc.tile_pool(name="small", bufs=2) as spool,
            tc.tile_pool(name="arep", bufs=1) as apool,
            tc.tile_pool(name="sig", bufs=1) as sigpool,
            tc.tile_pool(name="scr", bufs=2) as scrpool,
            tc.tile_pool(name="wrep", bufs=1) as wpool,
            tc.tile_pool(name="fus", bufs=1) as fuspool,
            tc.tile_pool(name="ftmp", bufs=1) as ftmppool,
            tc.tile_pool(name="yo", bufs=1) as ypool,
            tc.tile_pool(name="psT", bufs=1, space="PSUM") as psT,
            tc.tile_pool(name="psS", bufs=1, space="PSUM") as psS,
            tc.tile_pool(name="psB", bufs=1, space="PSUM") as psB,
        ):
            # --- static greedy balancing of elementwise work (DVE vs Pool).
            busy = {"v": 0.0, "p": 0.0}

            def pick(cost_v, cost_p):
                # walrus codegen rejects TensorTensor on Pool: DVE only.
                busy["v"] += cost_v
                return nc.vector

            def tt(out, in0, in1, op, nelem, is_addmul=False):
                cv = nelem * 0.521 + 62.0
                cp = nelem * (1.984 if is_addmul else 1.389) + 106.0
                eng = pick(cv, cp)
                eng.tensor_tensor(out=out, in0=in0, in1=in1, op=op)

            ones_col = cpool.tile([128, 1], F16, tag="ones_col")
            nc.vector.memset(ones_col[:], 1.0)
            ones_row = cpool.tile([1, 128], F16, tag="ones_row")
            nc.vector.memset(ones_row[:], 1.0)
            wft = {}
            for cc in range(2):
                for oc in range(2):
                    t = cpool.tile([128, 128], F16, tag=f"wft{cc}{oc}")
                    nc.sync.dma_start(
                        out=t[:],
                        in_=wft_h[cc * 128:(cc + 1) * 128, oc * 128:(oc + 1) * 128],
                    )
                    wft[(cc, oc)] = t
            bft = {}
            for oc in range(2):
                t = cpool.tile([128, 1], F32, tag=f"bf{oc}")
                nc.sync.dma_start(out=t[:], in_=bf_h[oc * 128:(oc + 1) * 128, :])
                bft[oc] = t

            def emit_loads(blk):
                off = blk * REG
                st = {}
                p1 = []
                for cc in range(2):
                    t = p1pool.tile([128, PBLK], F16, tag=f"p1_{cc}")
                    nc.sync.dma_start(
                        out=t[:, 0:PBLK],
                        in_=x1_h[cc * 128:(cc + 1) * 128, off:off + PBLK],
                    )
                    p1.append(t)
                p2 = []
                for n in range(N):
                    row = []
                    for cc in range(2):
                        t = p2pool.tile([128, PBLK], F16, tag=f"p2_{n}_{cc}")
                        nc.sync.dma_start(
                            out=t[:, 0:PBLK],
                            in_=x2_h[n, cc * 128:(cc + 1) * 128, off:off + PBLK],
                        )
                        row.append(t)
                    p2.append(row)
                st["p1"], st["p2"], st["off"] = p1, p2, off
                return st

            def emit_head(st):
                p1, p2 = st["p1"], st["p2"]

                # channel sums -> box filter -> avg (baseline-style, fp16)
                aflats = []
                for t5 in range(5):
                    src = p1 if t5 == 0 else p2[t5 - 1]
                    ps = psT.tile([1, 2048], F32, tag="tv")
                    for cc in range(2):
                        for (so, sl) in SUBS_SRC:
                            nc.tensor.matmul(
                                out=ps[0:1, so:so + sl],
                                lhsT=ones_col[:],
                                rhs=src[cc][:, so:so + sl],
                                start=(cc == 0),
                                stop=(cc == 1),
                            )
                    sflat = spool.tile([1, 1088], F32, tag=f"sflat{t5}", bufs=1)
                    nc.scalar.copy(out=sflat[0:1, 0:SRC_LEN], in_=ps[0:1, 0:SRC_LEN])
                    sA = spool.tile([RB, WP], F32, tag=f"sA{t5}", bufs=1)
                    sB = spool.tile([RB, WP], F32, tag=f"sB{t5}", bufs=1)
                    sC = spool.tile([RB, WP], F32, tag=f"sC{t5}", bufs=1)
                    nc.sync.dma_start(out=sA[:], in_=sflat[0:1, 0:RB * WP])
                    nc.sync.dma_start(out=sB[:], in_=sflat[0:1, WP:WP + RB * WP])
                    nc.sync.dma_start(out=sC[:], in_=sflat[0:1, 2 * WP:2 * WP + RB * WP])
                    tv2 = spool.tile([RB, WP], F32, tag=f"tv2{t5}", bufs=1)
                    nc.vector.tensor_add(out=tv2[:], in0=sA[:], in1=sB[:])
                    nc.vector.tensor_add(out=tv2[:], in0=tv2[:], in1=sC[:])
                    th = spool.tile([RB, WP], F32, tag=f"th{t5}", bufs=1)
                    nc.vector.tensor_add(out=th[:, 0:W], in0=tv2[:, 0:W], in1=tv2[:, 1:W + 1])
                    nc.vector.tensor_add(out=th[:, 0:W], in0=th[:, 0:W], in1=tv2[:, 2:W + 2])
                    av = spool.tile([RB, WP], F16, tag="av", bufs=1)
                    nc.vector.tensor_scalar(
                        out=av[:, 0:W], in0=th[:, 0:W], scalar1=INV9C,
                        scalar2=None, op0=mybir.AluOpType.mult,
                    )
                    nc.gpsimd.memset(av[:, W:WP], 0.0)
                    af = spool.tile([1, REG], F16, tag=f"aflat{t5}", bufs=1)
                    nc.sync.dma_start(out=af[0:1, 0:REG], in_=av[:])
                    aflats.append(af)

                def build_rep_flat(rowtile, tag, nbufs=1, pool=None):
                    ps = psB.tile([128, 770], F32, tag="psB")
                    for (so, sl) in SUBS:
                        nc.tensor.matmul(
                            out=ps[:, so:so + sl],
                            lhsT=ones_row[:],
                            rhs=rowtile[0:1, so:so + sl],
                            start=True,
                            stop=True,
                        )
                    rep = (pool or apool).tile([128, REG], F16, tag=tag, bufs=nbufs)
                    nc.scalar.copy(out=rep[:], in_=ps[:, 0:REG])
                    return rep

                a1rep = build_rep_flat(aflats[0], "a1rep")
                sig1 = []
                for cc in range(2):
                    sg = sigpool.tile([128, BIG], F16, tag=f"sig1_{cc}")
                    for dr in range(3):
                        tt(sg[:, dr * 3 * REG:(dr + 1) * 3 * REG],
                           _win3(p1[cc], BASE, dr), _rep3(a1rep), GE, 3 * REG)
                    sig1.append(sg)
                st["sig1"], st["build_rep_flat"], st["aflats"] = sig1, build_rep_flat, aflats
                return st

            def emit_head2(st):
                p1, p2 = st["p1"], st["p2"]
                sig1, build_rep_flat, aflats = st["sig1"], st["build_rep_flat"], st["aflats"]
                sim2d = []
                for n in range(N):
                    a2rep = build_rep_flat(aflats[1 + n], "a2rep", nbufs=2)
                    ps = psS.tile([1, 770], F32, tag="psS")
                    for cc in range(2):
                        for dr in range(3):
                            cv = 2 * (3 * REG * 0.521 + 62.0)
                            cp = 2 * (3 * REG * 1.389 + 106.0)
                            eng = pick(cv, cp)
                            sg2 = scrpool.tile([128, 3 * REG], F16, tag="sg2")
                            eng.tensor_tensor(out=sg2[:],
                                              in0=_win3(p2[n][cc], BASE, dr),
                                              in1=_rep3(a2rep), op=GE)
                            xn = scrpool.tile([128, 3 * REG], F16, tag="xn")
                            eng.tensor_tensor(
                                out=xn[:],
                                in0=sig1[cc][:, dr * 3 * REG:(dr + 1) * 3 * REG],
                                in1=sg2[:], op=EQ)
                            for dc in range(3):
                                for (so, sl) in SUBS:
                                    nc.tensor.matmul(
                                        out=ps[0:1, so:so + sl],
                                        lhsT=ones_col[:],
                                        rhs=xn[:, dc * REG + so:dc * REG + so + sl],
                                        start=(cc == 0 and dr == 0 and dc == 0),
                                        stop=(cc == 1 and dr == 2 and dc == 2),
                                    )
                    simflat = spool.tile([1, REG], F32, tag=f"simflat{n}", bufs=1)
                    nc.scalar.copy(out=simflat[:], in_=ps[0:1, 0:REG])
                    s2 = spool.tile([RB, WP], F32, tag=f"sim2d{n}")
                    nc.sync.dma_start(out=s2[:], in_=simflat[0:1, 0:REG])
                    sim2d.append(s2)
                st["sim2d"] = sim2d

            def emit_tail(st):
                p1, p2, off, sim2d = st["p1"], st["p2"], st["off"], st["sim2d"]
                build_rep_flat = st["build_rep_flat"]
                SM = WP
                mx = spool.tile([RB, WP], F32, tag="mx", bufs=1)
                tt(mx[:], sim2d[0][:], sim2d[1][:], mybir.AluOpType.max, SM)
                tt(mx[:], mx[:], sim2d[2][:], mybir.AluOpType.max, SM)
                tt(mx[:], mx[:], sim2d[3][:], mybir.AluOpType.max, SM)
                es = []
                for n in range(N):
                    d = spool.tile([RB, WP], F32, tag=f"ed{n}")
                    tt(d[:], sim2d[n][:], mx[:], mybir.AluOpType.subtract, SM,
                       is_addmul=True)
                    nc.scalar.activation(
                        out=d[:], in_=d[:], func=mybir.ActivationFunctionType.Exp
                    )
                    es.append(d)
                den = spool.tile([RB, WP], F32, tag="den", bufs=1)
                tt(den[:], es[0][:], es[1][:], mybir.AluOpType.add, SM,
                   is_addmul=True)
                tt(den[:], den[:], es[2][:], mybir.AluOpType.add, SM,
                   is_addmul=True)
                tt(den[:], den[:], es[3][:], mybir.AluOpType.add, SM,
                   is_addmul=True)
                rec = spool.tile([RB, WP], F32, tag="rec", bufs=1)
                nc.vector.reciprocal(out=rec[:], in_=den[:])
                busy["v"] += SM * 1.0417 + 62.0

                fused = []
                for cc in range(2):
                    fu = fuspool.tile([128, REG], F16, tag=f"fu{cc}")
                    nc.scalar.copy(out=fu[:], in_=p1[cc][:, BASE:BASE + REG])
                    fused.append(fu)
                for n in range(N):
                    wv = spool.tile([RB, WP], F16, tag=f"wv{n}")
                    tt(wv[:], es[n][:], rec[:], mybir.AluOpType.mult, SM,
                       is_addmul=True)
                    wf = spool.tile([1, REG], F16, tag=f"wflat{n}", bufs=1)
                    nc.sync.dma_start(out=wf[0:1, 0:REG], in_=wv[:])
                    wr = build_rep_flat(wf, f"wrep{n}", pool=wpool)
                    for cc in range(2):
                        tmp = ftmppool.tile([128, REG], F16, tag="ftmp")
                        tt(tmp[:], wr[:], p2[n][cc][:, BASE:BASE + REG],
                           mybir.AluOpType.mult, REG, is_addmul=True)
                        tt(fused[cc][:], fused[cc][:], tmp[:],
                           mybir.AluOpType.add, REG, is_addmul=True)

                for oc in range(2):
                    ps = psB.tile([128, 770], F32, tag="psB")
                    for cc in range(2):
                        for (so, sl) in SUBS:
                            nc.tensor.matmul(
                                out=ps[:, so:so + sl],
                                lhsT=wft[(cc, oc)][:],
                                rhs=fused[cc][:, so:so + sl],
                                start=(cc == 0),
                                stop=(cc == 1),
                            )
                    yo = ypool.tile([128, REG], F32, tag="yo")
                    nc.scalar.activation(
                        out=yo[:],
                        in_=ps[:, 0:REG],
                        func=mybir.ActivationFunctionType.Identity,
                        bias=bft[oc][:],
                        scale=1.0,
                    )
                    nc.sync.dma_start(
                        out=y_h[oc * 128:(oc + 1) * 128, off:off + REG], in_=yo[:]
                    )

            # software pipeline: census stream of block b runs ahead of the
            # softmax/fusion tail of block b-1
            prev = None
            for blk in range(NBLK):
                st = emit_loads(blk)
                emit_head(st)
                emit_head2(st)
                if prev is not None:
                    emit_tail(prev)
                prev = st
            emit_tail(prev)
    nc.compile()
    return nc


def get_nc():
    if "nc" not in _NC_CACHE:
        _NC_CACHE["nc"] = build_nc()
    return _NC_CACHE["nc"]


def shard_inputs(features, nearby_features, w_fuse, b_fuse):
    features = np.asarray(features, np.float32)
    nearby_features = np.asarray(nearby_features, np.float32)
    wft = np.ascontiguousarray(np.asarray(w_fuse, np.float16).T)
    bf = np.ascontiguousarray(np.asarray(b_fuse, np.float32).reshape(C, 1))
    cidx = np.clip(np.arange(-1, W + 1), 0, W - 1)
    in_maps = []
    for b in range(B):
        for half in range(2):
            h0 = half * HH
            ridx = np.clip(np.arange(h0 - 1, h0 + HH + 1), 0, H - 1)
            x1p = features[b][:, ridx][:, :, cidx].reshape(C, -1)
            x1 = np.zeros((C, XLEN), np.float16)
            x1[:, : x1p.shape[1]] = x1p
            x2p = nearby_features[b][:, :, ridx][:, :, :, cidx].reshape(N, C, -1)
            x2 = np.zeros((N, C, XLEN), np.float16)
            x2[:, :, : x2p.shape[2]] = x2p
            in_maps.append(
                {
                    "x1": np.ascontiguousarray(x1),
                    "x2": np.ascontiguousarray(x2),
                    "wft": wft,
                    "bf": bf,
                }
            )
    return in_maps


def gather_output(results):
    out = np.empty((B, C, H, W), np.float32)
    for i, r in enumerate(results):
        b, half = i // 2, i % 2
        y = np.asarray(r["y"]).reshape(C, HH, WP)[:, :, :W]
        out[b, :, half * HH:(half + 1) * HH, :] = y
    return out


def kernel(features, nearby_features, w_fuse, b_fuse, _trace=False, _trace_kwargs=None):
    in_maps = shard_inputs(features, nearby_features, w_fuse, b_fuse)
    nc = get_nc()
    kw = {}
    if _trace:
        kw = dict(trace=True, **(_trace_kwargs or {}))
    res = run_bass_kernel_spmd(nc, in_maps, core_ids=list(range(8)), **kw)
    out = gather_output(res.results)
    kernel._last_result = res
    return out
